# revision 16
# baseline (speedup 1.0000x reference)
"""DenseQAPNet Trainium2 kernel (8 NeuronCores, Bass/Tile).

Sharding: rows i of the n x n edge tensors across 8 cores (64 rows each).
The sum-over-j aggregation is row-local; one AllGather of the 4 aggregation
shards per conv layer; the (tiny) node-feature MLPs run redundantly on all
cores. The final cartesian pairwise MLP shards over rows of a.

Layout: activations live as [128 features (partitions), rows (free)] tiles.
Matmuls run in float32r (TF32-like). LeakyReLU is a single ScalarE Lrelu op
with fused per-partition bias, and for the last enc layer a fused free-dim
accum_out performs the sum-over-j aggregation.
"""

import numpy as np

import concourse.bass as bass
import concourse.mybir as mybir
import concourse.tile as tile
from concourse import bacc, bass_utils, dve_ops
from concourse.dve_spec import C0, C1, Spec, Src0, _has_src1, lower, maxx
from concourse.dve_uop import DveOpSpec

LEAKY_NAME = "LEAKY_BIAS_ANT"


def _leaky_ref(in0, in1, s0, s1, imm2):
    shp = in0.shape
    z = in0.astype(np.float32).reshape(shp[0], -1)
    if isinstance(s0, np.ndarray):
        s0 = s0.reshape(shp[0], -1)
    z = z + s0
    return np.maximum(z * s1, z).reshape(shp)


def _register_leaky():
    """Fused one-instruction DVE leaky-relu: out = max((in0+s0)*s1, in0+s0).
    Appended to the process-local custom-op registry; the compile path
    generates the per-NEFF uop table from it."""
    for op in dve_ops.OPS:
        if op.name == LEAKY_NAME:
            return op
    t = Src0 + C0
    spec = Spec(body=maxx(t * C1, t), reference=_leaky_ref)
    row = dve_ops._CUSTOM_DVE_ROW_BASE + len(dve_ops.OPS)
    assert row < 0x20
    dve_ops._SUB_OPCODE_FOR_NAME[LEAKY_NAME] = row
    shas = {}
    for ver in ("v3", "v4"):
        tmp = DveOpSpec(
            name=LEAKY_NAME, opcode=row, uops=lower(spec, ver=ver),
            rd1_en=_has_src1(spec),
        )
        shas[ver] = tmp.sha(ver)
    op = dve_ops.DveOp(LEAKY_NAME, spec, subdim=False, uops_sha=shas)
    dve_ops.OPS.append(op)
    dve_ops.CUSTOM_DVE_SPECS[LEAKY_NAME] = spec
    return op


_LEAKY_OP = _register_leaky()

F32 = mybir.dt.float32
F32R = mybir.dt.float32r
LRELU = mybir.ActivationFunctionType.Lrelu
SQUARE = mybir.ActivationFunctionType.Square
SQRT = mybir.ActivationFunctionType.Sqrt
ADD = mybir.AluOpType.add
SUB = mybir.AluOpType.subtract
MULT = mybir.AluOpType.mult
MAX = mybir.AluOpType.max
AXX = mybir.AxisListType.X

NN = 512          # graph nodes
W = 128           # feature width
P = 128           # partitions
NCORES = 8
S = NN // NCORES  # 64 rows per core
IB = 2            # i-rows per inner block
NBLK = S // IB    # 16 blocks
CONV_DEPTH = 3
SLOPE = 0.01
EPS = 1e-5
CA = 288          # h0 column split: [:CA] on ACT, [CA:] on DVE

# conv order within a layer: (branch, kind) -> edge tensor index in `edg` input
CONVS = [("a", "q"), ("a", "l"), ("b", "q"), ("b", "l")]
EDGE_IDX = {("a", "q"): 0, ("a", "l"): 1, ("b", "q"): 2, ("b", "l"): 3}


def _slots():
    """Weight-packing slot maps shared by host packing and device program."""
    wmat = {}   # name -> slot idx, each slot [128, 128] float32r
    wvec = {}   # name -> slot idx, each slot [1, 128] float32r
    bias = {}   # name -> column idx in [128, nb] float32

    def m(name):
        wmat[name] = len(wmat)

    def v(name):
        wvec[name] = len(wvec)

    def b(name):
        bias[name] = len(bias)

    for l in range(CONV_DEPTH):
        for br, cv in CONVS:
            tag = f"{l}{br}{cv}"
            if l > 0:
                m(f"nb_{tag}")
            m(f"w2_{tag}")
            m(f"w3_{tag}")
            m(f"wtr_{tag}")
            v(f"we_{tag}")
            b(f"b1_{tag}")
            b(f"b2_{tag}")
            b(f"b3_{tag}")
            b(f"btr_{tag}")
        for br in "ab":
            for k in range(3):
                m(f"cb{k}_{l}{br}")
                b(f"cbb{k}_{l}{br}")
    for name in ("w1a", "w1b", "w2L", "w3L", "w4"):
        m(name)
    v("wtilneg")
    for name in ("ctil", "b2L", "b3L", "epsv"):
        b(name)
    return wmat, wvec, bias


WMAT_SLOTS, WVEC_SLOTS, BIAS_SLOTS = _slots()
NM, NV, NB = len(WMAT_SLOTS), len(WVEC_SLOTS), len(BIAS_SLOTS)


def pack_weights(params):
    """Host-side packing of the params pytree into flat input tensors."""
    def np32(x):
        return np.asarray(x, dtype=np.float32)

    wmat = np.zeros((P, NM * P), np.float32)
    wvec = np.zeros((1, NV * P), np.float32)
    bias = np.zeros((P, NB), np.float32)
    scal = np.zeros((1, 2), np.float32)

    def putm(name, arr):  # arr [K<=128, M<=128], lhsT layout
        k, m = arr.shape
        wmat[:k, WMAT_SLOTS[name] * P : WMAT_SLOTS[name] * P + m] = arr

    def putv(name, arr):  # arr [<=128]
        wvec[0, WVEC_SLOTS[name] * P : WVEC_SLOTS[name] * P + arr.shape[0]] = arr

    def putb(name, arr):  # arr [<=128]
        bias[: arr.shape[0], BIAS_SLOTS[name]] = arr

    for l in range(CONV_DEPTH):
        for br, cv in CONVS:
            tag = f"{l}{br}{cv}"
            pp = params[br + str(l)]
            enc = pp[cv + "_enc"]
            (w1, b1), (w2, b2), (w3, b3) = [(np32(w), np32(b)) for w, b in enc]
            putv(f"we_{tag}", w1[0])
            putb(f"b2_{tag}", b2)
            if l > 0:
                putm(f"nb_{tag}", w1[1:])
            putm(f"w2_{tag}", w2)
            putm(f"w3_{tag}", w3)
            putb(f"b1_{tag}", b1)
            putb(f"b3_{tag}", b3)
            (wtr, btr), = [(np32(w), np32(b)) for w, b in pp[cv + "_tr"]]
            putm(f"wtr_{tag}", wtr)
            putb(f"btr_{tag}", btr)
        for br in "ab":
            for k, (w, b) in enumerate(params[br + str(l)]["comb"]):
                putm(f"cb{k}_{l}{br}", np32(w))
                putb(f"cbb{k}_{l}{br}", np32(b))

    g2, be = np32(params["pair_norm"][0]), np32(params["pair_norm"][1])
    (w1L, b1L), (w2L, b2L), (w3L, b3L), (w4, b4) = [
        (np32(w), np32(b)) for w, b in params["link"]
    ]
    putm("w1a", w1L[:W] * g2[:W, None])
    putm("w1b", w1L[W:] * g2[W:, None])
    putm("w2L", w2L)
    putm("w3L", w3L)
    putm("w4", w4)  # [128, 1]
    wtil = g2 @ w1L
    putv("wtilneg", -wtil)
    putb("ctil", be @ w1L + b1L)
    putb("b2L", b2L)
    putb("b3L", b3L)
    putb("epsv", np.full(P, EPS, np.float32))
    scal[0, 0] = float(np.asarray(b4).reshape(-1)[0])
    return wmat, wvec, bias, scal


def build_nc(with_collectives=True, debug_taps=False):
    nc = bacc.Bacc(
        "TRN2",
        target_bir_lowering=False,
        debug=False,
        enable_asserts=True,
        num_devices=NCORES if with_collectives else 1,
    )
    edg = nc.dram_tensor("edg", [4, S, NN], F32R, kind="ExternalInput").ap()
    wmat_d = nc.dram_tensor("wmat", [P, NM * P], F32R, kind="ExternalInput").ap()
    wvec_d = nc.dram_tensor("wvec", [1, NV * P], F32R, kind="ExternalInput").ap()
    bias_d = nc.dram_tensor("bias", [P, NB], F32, kind="ExternalInput").ap()
    scal_d = nc.dram_tensor("scal", [1, 2], F32, kind="ExternalInput").ap()
    ones_d = nc.dram_tensor("onesr", [1, NN], F32R, kind="ExternalInput").ap()
    onec_d = nc.dram_tensor("onesc", [P, 1], F32R, kind="ExternalInput").ap()
    out_d = nc.dram_tensor("out", [S, NN], F32, kind="ExternalOutput").ap()
    dbg = {}
    if debug_taps:
        for l in range(CONV_DEPTH):
            dbg[f"xsh{l}"] = nc.dram_tensor(
                f"dbg_xsh{l}", [4, P, S], F32, kind="ExternalOutput"
            ).ap()
            dbg[f"ab{l}"] = nc.dram_tensor(
                f"dbg_ab{l}", [2, P, NN], F32, kind="ExternalOutput"
            ).ap()
            dbg[f"ash{l}"] = nc.dram_tensor(
                f"dbg_ash{l}", [P, S], F32, kind="ExternalOutput"
            ).ap()
        dbg["mr"] = nc.dram_tensor("dbg_mr", [2, S, NN], F32, kind="ExternalOutput").ap()
        dbg["ut"] = nc.dram_tensor("dbg_ut", [S, P], F32, kind="ExternalOutput").ap()

    agin = [
        nc.dram_tensor(f"agin{l}", [4, P, S], F32, kind="Internal").ap()
        for l in range(CONV_DEPTH)
    ]
    agout = [
        nc.dram_tensor(
            f"agout{l}", [NCORES, 4, P, S], F32, kind="Internal", addr_space="Shared"
        ).ap()
        for l in range(CONV_DEPTH)
    ]
    # link-phase row staging (DRAM round-trip to re-layout rows to partition 0)
    utd = nc.dram_tensor("utd", [S, P], F32R, kind="Internal").ap()
    md_d = nc.dram_tensor("md", [S, NN], F32R, kind="Internal").ap()
    rd_d = nc.dram_tensor("rd", [S, NN], F32R, kind="Internal").ap()

    with tile.TileContext(nc) as tc:
        with (
            tc.tile_pool(name="wp", bufs=1) as wp,
            tc.tile_pool(name="npl", bufs=5) as npl,
            tc.tile_pool(name="xp", bufs=5) as xp,
            tc.tile_pool(name="wk", bufs=3) as wk,
            tc.tile_pool(name="eb", bufs=2) as ebp,
            tc.tile_pool(name="ps", bufs=4, space="PSUM") as ps,
        ):
            wmat = wp.tile([P, NM * P], F32R)
            biast = wp.tile([P, NB], F32)
            scal = wp.tile([1, 2], F32)
            onesr = wp.tile([1, NN], F32R)
            onesc = wp.tile([P, 1], F32R)
            wtn = wp.tile([1, P], F32R, name="wtn")
            nc.sync.dma_start(wmat[:], wmat_d)
            nc.sync.dma_start(
                wtn[:],
                wvec_d[:, WVEC_SLOTS["wtilneg"] * P : (WVEC_SLOTS["wtilneg"] + 1) * P],
            )
            nc.sync.dma_start(biast[:], bias_d)
            nc.sync.dma_start(scal[:], scal_d)
            nc.sync.dma_start(onesr[:], ones_d)
            nc.sync.dma_start(onesc[:], onec_d)

            def wm(name):  # [128,128] float32r lhsT slot
                return wmat[:, WMAT_SLOTS[name] * P : (WMAT_SLOTS[name] + 1) * P]

            def bi(name):  # [128,1] float32
                return biast[:, BIAS_SLOTS[name] : BIAS_SLOTS[name] + 1]

            ones_f = onesr.bitcast(F32)
            onesc_f = onesc.bitcast(F32)

            def edge_layer(l, nb_for, x_shs):
                """All 4 convs of layer l, software-pipelined at depth 2 over
                (conv, ib) work items so every engine's in-order stream
                interleaves stages of neighbouring items."""
                wvts = {}
                for br, cv in CONVS:
                    tag = f"{l}{br}{cv}"
                    wvt = wk.tile([1, P], F32R, name=f"wvt_{tag}", tag="wvt4")
                    s0 = WVEC_SLOTS[f"we_{tag}"]
                    nc.sync.dma_start(wvt[:], wvec_d[:, s0 * P : (s0 + 1) * P])
                    wvts[(br, cv)] = wvt
                items = [(br, cv, ib) for br, cv in CONVS for ib in range(NBLK)]
                live = {}

                def s0_stage(it):
                    br, cv, ib = it
                    tag = f"{l}{br}{cv}"
                    eidx = EDGE_IDX[(br, cv)]
                    eblk = ebp.tile([1, IB, NN], F32R, name=f"e{tag}_{ib}", tag="eblk")
                    nc.sync.dma_start(
                        eblk[:].rearrange("a b c -> a (b c)"),
                        edg[eidx, ib * IB : (ib + 1) * IB, :]
                        .rearrange("a b -> (a b)")[None, :],
                    )
                    pz0 = ps.tile([P, IB, NN], F32, name=f"z0{tag}{ib}", tag="eps")
                    nb_tile = nb_for(br, cv)
                    for k in range(IB):
                        if nb_tile is not None:
                            nc.tensor.matmul(
                                pz0[:, k, :], wm(f"nb_{tag}"), nb_tile[:],
                                start=True, stop=False,
                            )
                            nc.tensor.matmul(
                                pz0[:, k, :], wvts[(br, cv)][:], eblk[0:1, k, :],
                                start=False, stop=True,
                            )
                        else:
                            nc.tensor.matmul(
                                pz0[:, k, :], wvts[(br, cv)][:], eblk[0:1, k, :],
                                start=True, stop=True,
                            )
                    live[("z0", it)] = pz0

                def s1_stage(it):
                    br, cv, ib = it
                    tag = f"{l}{br}{cv}"
                    pz0 = live.pop(("z0", it))
                    h0 = wk.tile([P, IB, NN], F32R, name=f"h0{tag}{ib}", tag="ha")
                    nc.scalar.activation(
                        h0[:, :, :CA], pz0[:, :, :CA], LRELU,
                        bias=bi(f"b1_{tag}"), scale=1.0, alpha=SLOPE,
                    )
                    nc.vector._custom_dve(
                        _LEAKY_OP, out=h0[:, :, CA:], in0=pz0[:, :, CA:],
                        s0=bi(f"b1_{tag}"), s1=SLOPE,
                    )
                    pz1 = ps.tile([P, IB, NN], F32, name=f"z1{tag}{ib}", tag="eps")
                    for k in range(IB):
                        nc.tensor.matmul(
                            pz1[:, k, :], wm(f"w2_{tag}"), h0[:, k, :],
                            start=True, stop=True,
                        )
                    live[("z1", it)] = pz1

                def s2_stage(it):
                    br, cv, ib = it
                    tag = f"{l}{br}{cv}"
                    pz1 = live.pop(("z1", it))
                    h1 = wk.tile([P, IB, NN], F32R, name=f"h1{tag}{ib}", tag="hb")
                    nc.vector._custom_dve(
                        _LEAKY_OP, out=h1[:], in0=pz1[:], s0=bi(f"b2_{tag}"),
                        s1=SLOPE,
                    )
                    pz2 = ps.tile([P, IB, NN], F32, name=f"z2{tag}{ib}", tag="eps")
                    for k in range(IB):
                        nc.tensor.matmul(
                            pz2[:, k, :], wm(f"w3_{tag}"), h1[:, k, :],
                            start=True, stop=True,
                        )
                    live[("z2", it)] = pz2

                def s3_stage(it):
                    br, cv, ib = it
                    tag = f"{l}{br}{cv}"
                    pz2 = live.pop(("z2", it))
                    junk = wk.tile([P, IB, NN], F32, name=f"j{tag}{ib}", tag="u01")
                    x_sh = x_shs[(br, cv)]
                    for k in range(IB):
                        i = ib * IB + k
                        nc.scalar.activation(
                            junk[:, k, :], pz2[:, k, :], LRELU,
                            bias=bi(f"b3_{tag}"), scale=1.0, alpha=SLOPE,
                            accum_out=x_sh[:, i : i + 1],
                        )

                stages = [s0_stage, s1_stage, s2_stage, s3_stage]
                nst = len(stages)
                for t in range(len(items) + nst - 1):
                    for si in range(nst - 1, -1, -1):
                        j = t - si
                        if 0 <= j < len(items):
                            stages[si](items[j])

            def global_ln_apply(x_full, xhat, tagn):
                """xhat = (x - mean)*rsqrt(var+eps) over the whole [128,512]
                tensor; returns the [128,2] (m,r) broadcast tile for reuse."""
                s12 = wk.tile([P, 2], F32, name=f"s12_{tagn}", tag="s12")
                nc.vector.reduce_sum(s12[:, 0:1], x_full[:], axis=AXX)
                xsq = wk.tile([P, NN], F32, name=f"xsq_{tagn}", tag="u01")
                nc.scalar.activation(
                    xsq[:], x_full[:], SQUARE, bias=0.0, scale=1.0,
                    accum_out=s12[:, 1:2],
                )
                pst = ps.tile([1, 2], F32, name=f"pst_{tagn}", tag="eps")
                nc.tensor.matmul(pst[:], onesc_f, s12[:], start=True, stop=True)
                sc = wk.tile([1, 4], F32, name=f"sc_{tagn}", tag="scs")
                # sc[0]=m, sc[1]=E[x^2]
                nc.vector.tensor_scalar(
                    sc[0:1, 0:2], pst[:], 1.0 / (NN * W), None, MULT
                )
                # sc[2] = var = E[x^2] - m^2 ; sc[3] = sd = sqrt(var + eps)
                nc.scalar.activation(sc[0:1, 2:3], sc[0:1, 0:1], SQUARE)
                nc.vector.tensor_tensor(
                    sc[0:1, 2:3], sc[0:1, 1:2], sc[0:1, 2:3], SUB
                )
                nc.scalar.activation(
                    sc[0:1, 3:4], sc[0:1, 2:3], SQRT, bias=bi("epsv")[0:1, :]
                )
                rin = wk.tile([1, 2], F32, name=f"rin_{tagn}", tag="scs2")
                nc.vector.tensor_copy(rin[0:1, 0:1], sc[0:1, 0:1])  # m
                nc.vector.reciprocal(rin[0:1, 1:2], sc[0:1, 3:4])   # r
                pbc = ps.tile([P, 2], F32, name=f"pbc_{tagn}", tag="eps")
                nc.tensor.matmul(
                    pbc[:], ones_f[0:1, 0:P], rin[:], start=True, stop=True
                )
                mr = npl.tile([P, 2], F32, name=f"mr_{tagn}", tag="mrbc")
                nc.scalar.copy(mr[:], pbc[:])
                nc.vector.tensor_scalar(
                    xhat[:], x_full[:], mr[:, 0:1], mr[:, 1:2], SUB, MULT
                )
                return mr

            def tr_layer(l, br, cv, xhat, out_t):
                tag = f"{l}{br}{cv}"
                pt = ps.tile(
                    [P, xhat.shape[-1]], F32, name=f"ptr_{out_t.name}", tag="eps"
                )
                nc.tensor.matmul(
                    pt[:], wm(f"wtr_{tag}"), xhat[:], start=True, stop=True
                )
                nc.scalar.activation(
                    out_t[:], pt[:], LRELU, bias=bi(f"btr_{tag}"), scale=1.0,
                    alpha=SLOPE,
                )

            def comb_mlp(l, br, c_t, width, sfx):
                h = c_t
                for k in range(3):
                    pc = ps.tile(
                        [P, width], F32, name=f"pcb{k}_{l}{br}{sfx}", tag="eps"
                    )
                    nc.tensor.matmul(
                        pc[:], wm(f"cb{k}_{l}{br}"), h[:], start=True, stop=True
                    )
                    if k == 2:
                        hn = npl.tile(
                            [P, width], F32R,
                            name=f"{br}{l + 1}{sfx}", tag=f"nodes{sfx}",
                        )
                    else:
                        hn = wk.tile(
                            [P, width], F32R,
                            name=f"ch{k}_{l}{br}{sfx}", tag=f"combh{sfx}",
                        )
                    nc.scalar.activation(
                        hn[:], pc[:], LRELU, bias=bi(f"cbb{k}_{l}{br}"),
                        scale=1.0, alpha=SLOPE,
                    )
                    h = hn
                return h

            # ---------------- conv layers ----------------
            a_cur = b_cur = None      # [128, 512] float32r full node features
            a_sh = None               # [128, S] float32r shard chain (a only)
            for l in range(CONV_DEPTH):
                x_shs = {}
                for br, cv in CONVS:
                    x_shs[(br, cv)] = xp.tile(
                        [P, S], F32, name=f"xsh_{l}{br}{cv}", tag="xsh"
                    )

                def nb_for(br, cv, _l=l, _a=a_cur, _b=b_cur):
                    if _l == 0:
                        return None
                    return {"q": {"a": _a, "b": _b},
                            "l": {"a": _b, "b": _a}}[cv][br]

                edge_layer(l, nb_for, x_shs)
                for ci, (br, cv) in enumerate(CONVS):
                    nc.sync.dma_start(agin[l][ci], x_shs[(br, cv)][:])
                if with_collectives:
                    nc.gpsimd.collective_compute(
                        "AllGather",
                        mybir.AluOpType.bypass,
                        replica_groups=[list(range(NCORES))],
                        ins=[agin[l].opt()],
                        outs=[agout[l].opt()],
                    )
                else:
                    nc.sync.dma_start(agout[l][0], agin[l])

                # node phase (redundant on all cores)
                t_full = {}
                mrs = {}
                for ci, (br, cv) in enumerate(CONVS):
                    tag = f"{l}{br}{cv}"
                    x_full = xp.tile([P, NN], F32, name=f"xf_{tag}", tag="xfull")
                    nc.sync.dma_start(
                        x_full[:].rearrange("f (c i) -> f c i", c=NCORES),
                        agout[l][:, ci, :, :].rearrange("c f i -> f c i"),
                    )
                    xhat = wk.tile([P, NN], F32R, name=f"xh_{tag}", tag="hb")
                    mrs[(br, cv)] = global_ln_apply(x_full, xhat, tag)
                    t_t = npl.tile([P, NN], F32R, name=f"t_{tag}", tag="tt")
                    tr_layer(l, br, cv, xhat, t_t)
                    t_full[(br, cv)] = t_t
                new_nodes = {}
                for br in "ab":
                    c_t = npl.tile([P, NN], F32R, name=f"c_{l}{br}", tag="nodes")
                    nc.vector.tensor_tensor(
                        c_t[:],
                        t_full[(br, "q")].bitcast(F32)[:],
                        t_full[(br, "l")].bitcast(F32)[:],
                        ADD,
                    )
                    if l > 0:
                        prev = a_cur if br == "a" else b_cur
                        nc.vector.tensor_tensor(
                            c_t[:], c_t.bitcast(F32)[:],
                            prev.bitcast(F32)[:], ADD,
                        )
                    new_nodes[br] = comb_mlp(l, br, c_t, NN, "")
                # shard chain for branch a (link phase needs per-core rows of a)
                xhq = wk.tile([P, S], F32R, name=f"xhq_sh{l}", tag="shs")
                mr_q = mrs[("a", "q")]
                nc.vector.tensor_scalar(
                    xhq[:], x_shs[("a", "q")][:], mr_q[:, 0:1], mr_q[:, 1:2],
                    SUB, MULT,
                )
                tq_sh = wk.tile([P, S], F32R, name=f"tq_sh{l}", tag="sht")
                tr_layer(l, "a", "q", xhq, tq_sh)
                xhl = wk.tile([P, S], F32R, name=f"xhl_sh{l}", tag="shs")
                mr_l = mrs[("a", "l")]
                nc.vector.tensor_scalar(
                    xhl[:], x_shs[("a", "l")][:], mr_l[:, 0:1], mr_l[:, 1:2],
                    SUB, MULT,
                )
                tl_sh = wk.tile([P, S], F32R, name=f"tl_sh{l}", tag="sht2")
                tr_layer(l, "a", "l", xhl, tl_sh)
                c_sh = npl.tile([P, S], F32R, name=f"csh_{l}", tag="nodesh")
                nc.vector.tensor_tensor(
                    c_sh[:], tq_sh.bitcast(F32)[:],
                    tl_sh.bitcast(F32)[:], ADD,
                )
                if l > 0:
                    nc.vector.tensor_tensor(
                        c_sh[:], c_sh.bitcast(F32)[:],
                        a_sh.bitcast(F32)[:], ADD,
                    )
                a_sh = comb_mlp(l, "a", c_sh, S, "sh")
                a_cur, b_cur = new_nodes["a"], new_nodes["b"]
                if debug_taps:
                    for ci, (br, cv) in enumerate(CONVS):
                        nc.sync.dma_start(dbg[f"xsh{l}"][ci], x_shs[(br, cv)][:])
                    nc.sync.dma_start(dbg[f"ab{l}"][0], a_cur.bitcast(F32)[:])
                    nc.sync.dma_start(dbg[f"ab{l}"][1], b_cur.bitcast(F32)[:])
                    nc.sync.dma_start(dbg[f"ash{l}"], a_sh.bitcast(F32)[:])

            # ---------------- link (pairwise) phase ----------------
            # UT shard [S, 128] = a_sh.T @ W1a' (gamma folded on host)
            put = ps.tile([S, P], F32, name="put", tag="eps")
            nc.tensor.matmul(put[:], a_sh[:, :], wm("w1a"), start=True, stop=True)
            ut_sb = wp.tile([S, P], F32R, name="ut_sb", tag="ut")
            nc.scalar.copy(ut_sb[:], put[:])
            nc.sync.dma_start(utd, ut_sb[:])

            # row sums: sa/qa2 over shard a, sb/qb2 over full b (fp32 matmuls)
            prow_a = ps.tile([1, 2, S], F32, name="prow_a", tag="eps")
            nc.tensor.matmul(
                prow_a[:, 0, :], onesc_f, a_sh.bitcast(F32)[:],
                start=True, stop=True,
            )
            asq = wk.tile([P, S], F32, name="asq", tag="shs")
            nc.scalar.activation(asq[:], a_sh.bitcast(F32)[:], SQUARE)
            nc.tensor.matmul(
                prow_a[:, 1, :], onesc_f, asq[:], start=True, stop=True
            )
            sa_sb = wp.tile([1, 2, S], F32, name="sa_sb", tag="sarow")
            nc.scalar.copy(sa_sb[:], prow_a[:])

            prow_b0 = ps.tile([1, NN], F32, name="prow_b0", tag="eps")
            nc.tensor.matmul(
                prow_b0[:], onesc_f, b_cur.bitcast(F32)[:], start=True, stop=True
            )
            bsq = wk.tile([P, NN], F32, name="bsq", tag="u01")
            nc.scalar.activation(bsq[:], b_cur.bitcast(F32)[:], SQUARE)
            prow_b1 = ps.tile([1, NN], F32, name="prow_b1", tag="eps")
            nc.tensor.matmul(prow_b1[:], onesc_f, bsq[:], start=True, stop=True)
            sb_sb = wp.tile([1, 2, NN], F32, name="sb_sb", tag="sbrow")
            nc.scalar.copy(sb_sb[:, 0, :], prow_b0[:])
            nc.scalar.copy(sb_sb[:, 1, :], prow_b1[:])

            # M, Q [S, 512] ; then R = 1/sqrt(Q/256 - (M/256)^2 + eps)
            pmqs = []
            for t in range(2):
                pmq_t = ps.tile([S, NN], F32, name=f"pmq{t}", tag="eps")
                nc.tensor.matmul(
                    pmq_t[:], sa_sb[0:1, t, :], ones_f[:], start=True, stop=False
                )
                nc.tensor.matmul(
                    pmq_t[:], ones_f[0:1, 0:S], sb_sb[0:1, t, :],
                    start=False, stop=True,
                )
                pmqs.append(pmq_t)
            m_sb = wk.tile([S, NN], F32R, name="m_sb", tag="mlink")
            nc.vector.tensor_scalar(
                m_sb[:], pmqs[0][:], 1.0 / (2 * W), None, MULT
            )
            qn = wk.tile([S, NN], F32, name="qn", tag="ha")
            nc.vector.tensor_scalar(
                qn[:], pmqs[1][:], 1.0 / (2 * W), None, MULT
            )
            msq = wk.tile([S, NN], F32, name="msq", tag="hb")
            nc.scalar.activation(msq[:], m_sb.bitcast(F32)[:], SQUARE)
            nc.vector.tensor_tensor(qn[:], qn[:], msq[:], SUB)
            sd = wk.tile([S, NN], F32, name="sd", tag="hb")
            nc.scalar.activation(sd[:], qn[:], SQRT, bias=bi("epsv")[0:S, :])
            r_sb = wk.tile([S, NN], F32, name="r_sb", tag="rlink")
            nc.vector.reciprocal(r_sb[:], sd[:])
            nc.sync.dma_start(md_d, m_sb[:])
            nc.gpsimd.dma_start(rd_d, r_sb[:])  # fp32 -> f32r cast to DRAM
            if debug_taps:
                nc.sync.dma_start(dbg["mr"][0], m_sb.bitcast(F32)[:])
                nc.sync.dma_start(dbg["mr"][1], r_sb[:])
                nc.sync.dma_start(dbg["ut"], ut_sb.bitcast(F32)[:])

            lv = {}

            def l_s0(ib):
                ublk = ebp.tile([1, IB, P], F32R, name=f"ub{ib}", tag="ublk")
                nc.sync.dma_start(
                    ublk[:].rearrange("a b c -> a (b c)"),
                    utd[ib * IB : (ib + 1) * IB, :].rearrange("a b -> (a b)")[None, :],
                )
                mblk = ebp.tile([1, IB, NN], F32R, name=f"mb{ib}", tag="mblk")
                nc.sync.dma_start(
                    mblk[:].rearrange("a b c -> a (b c)"),
                    md_d[ib * IB : (ib + 1) * IB, :].rearrange("a b -> (a b)")[None, :],
                )
                rblk = ebp.tile([1, IB, NN], F32R, name=f"rb{ib}", tag="rblk")
                nc.sync.dma_start(
                    rblk[:].rearrange("a b c -> a (b c)"),
                    rd_d[ib * IB : (ib + 1) * IB, :].rearrange("a b -> (a b)")[None, :],
                )
                pr = ps.tile([P, IB, NN], F32, name=f"pr{ib}", tag="eps")
                for k in range(IB):
                    nc.tensor.matmul(
                        pr[:, k, :], onesr[0:1, 0:P], rblk[0:1, k, :],
                        start=True, stop=True,
                    )
                rrep = wk.tile([P, IB, NN], F32, name=f"rrep{ib}", tag="u01")
                nc.scalar.copy(rrep[:], pr[:])
                lv[("blk", ib)] = (ublk, mblk, rblk, rrep)

            def l_s1(ib):
                ublk, mblk, rblk, rrep = lv[("blk", ib)]
                pS = ps.tile([P, IB, NN], F32, name=f"pS{ib}", tag="eps")
                for k in range(IB):
                    nc.tensor.matmul(
                        pS[:, k, :], wm("w1b"), b_cur[:], start=True, stop=False
                    )
                    nc.tensor.matmul(
                        pS[:, k, :], ublk[0:1, k, :], onesr[:],
                        start=False, stop=False,
                    )
                    nc.tensor.matmul(
                        pS[:, k, :], wtn[:], mblk[0:1, k, :],
                        start=False, stop=True,
                    )
                z1s = wk.tile([P, IB, NN], F32, name=f"z1s{ib}", tag="z1s")
                nc.vector.tensor_tensor(z1s[:], pS[:], rrep[:], MULT)
                h1L = wk.tile([P, IB, NN], F32R, name=f"h1L{ib}", tag="ha")
                nc.scalar.activation(
                    h1L[:], z1s[:], LRELU, bias=bi("ctil"), scale=1.0, alpha=SLOPE
                )
                lv[("h1", ib)] = h1L

            def l_s2(ib):
                h1L = lv.pop(("h1", ib))
                pz2 = ps.tile([P, IB, NN], F32, name=f"pz2L{ib}", tag="eps")
                for k in range(IB):
                    nc.tensor.matmul(
                        pz2[:, k, :], wm("w2L"), h1L[:, k, :], start=True, stop=True
                    )
                h2L = wk.tile([P, IB, NN], F32R, name=f"h2L{ib}", tag="hb")
                nc.scalar.activation(
                    h2L[:], pz2[:], LRELU, bias=bi("b2L"), scale=1.0, alpha=SLOPE
                )
                lv[("h2", ib)] = h2L

            def l_s3(ib):
                h2L = lv.pop(("h2", ib))
                pz3 = ps.tile([P, IB, NN], F32, name=f"pz3L{ib}", tag="eps")
                for k in range(IB):
                    nc.tensor.matmul(
                        pz3[:, k, :], wm("w3L"), h2L[:, k, :], start=True, stop=True
                    )
                h3L = wk.tile([P, IB, NN], F32R, name=f"h3L{ib}", tag="u01")
                nc.vector._custom_dve(
                    _LEAKY_OP, out=h3L[:], in0=pz3[:], s0=bi("b3L"), s1=SLOPE
                )
                lv[("h3", ib)] = h3L

            def l_s4(ib):
                h3L = lv.pop(("h3", ib))
                po = ps.tile([1, IB, NN], F32, name=f"po{ib}", tag="eps")
                for k in range(IB):
                    nc.tensor.matmul(
                        po[0:1, k, :], wm("w4")[:, 0:1], h3L[:, k, :],
                        start=True, stop=True,
                    )
                oro = wk.tile([1, IB, NN], F32, name=f"oro{ib}", tag="mlink")
                nc.vector.tensor_scalar(oro[:], po[:], scal[0:1, 0:1], None, ADD)
                nc.sync.dma_start(
                    out_d[ib * IB : (ib + 1) * IB, :].rearrange("a b -> (a b)")[None, :],
                    oro[:].rearrange("a b c -> a (b c)"),
                )
                lv.pop(("blk", ib))

            lstages = [l_s0, l_s1, l_s2, l_s3, l_s4]
            for t in range(NBLK + len(lstages) - 1):
                for si in range(len(lstages) - 1, -1, -1):
                    j = t - si
                    if 0 <= j < NBLK:
                        lstages[si](j)

    nc.compile()
    return nc


_NC_CACHE = {}


def _prep_inputs(A, B, L, params):
    A = np.asarray(A, np.float32)
    B = np.asarray(B, np.float32)
    L = np.asarray(L, np.float32)
    wmat, wvec, bias, scal = pack_weights(params)
    onesr = np.ones((1, NN), np.float32)
    onesc = np.ones((P, 1), np.float32)
    LT = np.ascontiguousarray(L.T)
    in_maps = []
    for c in range(NCORES):
        rows = slice(c * S, (c + 1) * S)
        edg = np.stack([A[rows], L[rows], B[rows], LT[rows]], axis=0)
        in_maps.append(
            {
                "edg": np.ascontiguousarray(edg),
                "wmat": wmat,
                "wvec": wvec,
                "bias": bias,
                "scal": scal,
                "onesr": onesr,
                "onesc": onesc,
            }
        )
    return in_maps


def kernel(A, B, L, params):
    if "nc" not in _NC_CACHE:
        _NC_CACHE["nc"] = build_nc(with_collectives=True)
    nc = _NC_CACHE["nc"]
    in_maps = _prep_inputs(A, B, L, params)
    res = bass_utils.run_bass_kernel_spmd(nc, in_maps, core_ids=list(range(NCORES)))
    out = np.concatenate([res.results[c]["out"] for c in range(NCORES)], axis=0)
    return out.astype(np.float32)


# revision 17
# speedup vs baseline: 2.0106x; 2.0106x over previous
"""DenseQAPNet Trainium2 kernel (8 NeuronCores, Bass/Tile).

Sharding: rows i of the n x n edge tensors across 8 cores (64 rows each).
The sum-over-j aggregation is row-local; one AllGather of the 4 aggregation
shards per conv layer; the (tiny) node-feature MLPs run redundantly on all
cores. The final cartesian pairwise MLP shards over rows of a.

Layout: activations live as [128 features (partitions), rows (free)] tiles.
Matmuls run in float32r (TF32-like). LeakyReLU is a single ScalarE Lrelu op
with fused per-partition bias, and for the last enc layer a fused free-dim
accum_out performs the sum-over-j aggregation.
"""

import numpy as np

import concourse.bass as bass
import concourse.mybir as mybir
import concourse.tile as tile
from concourse import bacc, bass_utils, dve_ops
from concourse.dve_spec import C0, C1, Spec, Src0, _has_src1, lower, maxx
from concourse.dve_uop import DveOpSpec

LEAKY_NAME = "LEAKY_BIAS_ANT"


def _leaky_ref(in0, in1, s0, s1, imm2):
    shp = in0.shape
    z = in0.astype(np.float32).reshape(shp[0], -1)
    if isinstance(s0, np.ndarray):
        s0 = s0.reshape(shp[0], -1)
    z = z + s0
    return np.maximum(z * s1, z).reshape(shp)


def _register_leaky():
    """Fused one-instruction DVE leaky-relu: out = max((in0+s0)*s1, in0+s0).
    Appended to the process-local custom-op registry; the compile path
    generates the per-NEFF uop table from it."""
    for op in dve_ops.OPS:
        if op.name == LEAKY_NAME:
            return op
    t = Src0 + C0
    spec = Spec(body=maxx(t * C1, t), reference=_leaky_ref)
    row = dve_ops._CUSTOM_DVE_ROW_BASE + len(dve_ops.OPS)
    assert row < 0x20
    dve_ops._SUB_OPCODE_FOR_NAME[LEAKY_NAME] = row
    shas = {}
    for ver in ("v3", "v4"):
        tmp = DveOpSpec(
            name=LEAKY_NAME, opcode=row, uops=lower(spec, ver=ver),
            rd1_en=_has_src1(spec),
        )
        shas[ver] = tmp.sha(ver)
    op = dve_ops.DveOp(LEAKY_NAME, spec, subdim=False, uops_sha=shas)
    dve_ops.OPS.append(op)
    dve_ops.CUSTOM_DVE_SPECS[LEAKY_NAME] = spec
    return op


_LEAKY_OP = _register_leaky()

F32 = mybir.dt.float32
F32R = mybir.dt.float32r
LRELU = mybir.ActivationFunctionType.Lrelu
SQUARE = mybir.ActivationFunctionType.Square
SQRT = mybir.ActivationFunctionType.Sqrt
ADD = mybir.AluOpType.add
SUB = mybir.AluOpType.subtract
MULT = mybir.AluOpType.mult
MAX = mybir.AluOpType.max
AXX = mybir.AxisListType.X

NN = 512          # graph nodes
W = 128           # feature width
P = 128           # partitions
NCORES = 8
S = NN // NCORES  # 64 rows per core
IB = 2            # i-rows per inner block
NBLK = S // IB    # 16 blocks
CONV_DEPTH = 3
SLOPE = 0.01
EPS = 1e-5
CA = 288          # h0 column split: [:CA] on ACT, [CA:] on DVE

# conv order within a layer: (branch, kind) -> edge tensor index in `edg` input
CONVS = [("a", "q"), ("a", "l"), ("b", "q"), ("b", "l")]
EDGE_IDX = {("a", "q"): 0, ("a", "l"): 1, ("b", "q"): 2, ("b", "l"): 3}


def _slots():
    """Weight-packing slot maps shared by host packing and device program."""
    wmat = {}   # name -> slot idx, each slot [128, 128] float32r
    wvec = {}   # name -> slot idx, each slot [1, 128] float32r
    bias = {}   # name -> column idx in [128, nb] float32

    def m(name):
        wmat[name] = len(wmat)

    def v(name):
        wvec[name] = len(wvec)

    def b(name):
        bias[name] = len(bias)

    for l in range(CONV_DEPTH):
        for br, cv in CONVS:
            tag = f"{l}{br}{cv}"
            if l > 0:
                m(f"nb_{tag}")
            m(f"w2_{tag}")
            m(f"w3_{tag}")
            m(f"wtr_{tag}")
            v(f"we_{tag}")
            b(f"b1_{tag}")
            b(f"b2_{tag}")
            b(f"b3_{tag}")
            b(f"btr_{tag}")
        for br in "ab":
            for k in range(3):
                m(f"cb{k}_{l}{br}")
                b(f"cbb{k}_{l}{br}")
    for name in ("w1a", "w1b", "w2L", "w3L", "w4"):
        m(name)
    v("wtilneg")
    for name in ("ctil", "b2L", "b3L", "epsv"):
        b(name)
    return wmat, wvec, bias


WMAT_SLOTS, WVEC_SLOTS, BIAS_SLOTS = _slots()
NM, NV, NB = len(WMAT_SLOTS), len(WVEC_SLOTS), len(BIAS_SLOTS)


def pack_weights(params):
    """Host-side packing of the params pytree into flat input tensors."""
    def np32(x):
        return np.asarray(x, dtype=np.float32)

    wmat = np.zeros((P, NM * P), np.float32)
    wvec = np.zeros((1, NV * P), np.float32)
    bias = np.zeros((P, NB), np.float32)
    scal = np.zeros((1, 2), np.float32)

    def putm(name, arr):  # arr [K<=128, M<=128], lhsT layout
        k, m = arr.shape
        wmat[:k, WMAT_SLOTS[name] * P : WMAT_SLOTS[name] * P + m] = arr

    def putv(name, arr):  # arr [<=128]
        wvec[0, WVEC_SLOTS[name] * P : WVEC_SLOTS[name] * P + arr.shape[0]] = arr

    def putb(name, arr):  # arr [<=128]
        bias[: arr.shape[0], BIAS_SLOTS[name]] = arr

    for l in range(CONV_DEPTH):
        for br, cv in CONVS:
            tag = f"{l}{br}{cv}"
            pp = params[br + str(l)]
            enc = pp[cv + "_enc"]
            (w1, b1), (w2, b2), (w3, b3) = [(np32(w), np32(b)) for w, b in enc]
            putv(f"we_{tag}", w1[0])
            putb(f"b2_{tag}", b2)
            if l > 0:
                putm(f"nb_{tag}", w1[1:])
            putm(f"w2_{tag}", w2)
            putm(f"w3_{tag}", w3)
            putb(f"b1_{tag}", b1)
            putb(f"b3_{tag}", b3)
            (wtr, btr), = [(np32(w), np32(b)) for w, b in pp[cv + "_tr"]]
            putm(f"wtr_{tag}", wtr)
            putb(f"btr_{tag}", btr)
        for br in "ab":
            for k, (w, b) in enumerate(params[br + str(l)]["comb"]):
                putm(f"cb{k}_{l}{br}", np32(w))
                putb(f"cbb{k}_{l}{br}", np32(b))

    g2, be = np32(params["pair_norm"][0]), np32(params["pair_norm"][1])
    (w1L, b1L), (w2L, b2L), (w3L, b3L), (w4, b4) = [
        (np32(w), np32(b)) for w, b in params["link"]
    ]
    putm("w1a", w1L[:W] * g2[:W, None])
    putm("w1b", w1L[W:] * g2[W:, None])
    putm("w2L", w2L)
    putm("w3L", w3L)
    putm("w4", w4)  # [128, 1]
    wtil = g2 @ w1L
    putv("wtilneg", -wtil)
    putb("ctil", be @ w1L + b1L)
    putb("b2L", b2L)
    putb("b3L", b3L)
    putb("epsv", np.full(P, EPS, np.float32))
    scal[0, 0] = float(np.asarray(b4).reshape(-1)[0])
    return wmat, wvec, bias, scal


def build_nc(with_collectives=True, debug_taps=False):
    nc = bacc.Bacc(
        "TRN2",
        target_bir_lowering=False,
        debug=False,
        enable_asserts=True,
        num_devices=NCORES if with_collectives else 1,
    )
    edg = nc.dram_tensor("edg", [4, S, NN], F32R, kind="ExternalInput").ap()
    wmat_d = nc.dram_tensor("wmat", [P, NM * P], F32R, kind="ExternalInput").ap()
    wvec_d = nc.dram_tensor("wvec", [1, NV * P], F32R, kind="ExternalInput").ap()
    bias_d = nc.dram_tensor("bias", [P, NB], F32, kind="ExternalInput").ap()
    scal_d = nc.dram_tensor("scal", [1, 2], F32, kind="ExternalInput").ap()
    ones_d = nc.dram_tensor("onesr", [1, NN], F32R, kind="ExternalInput").ap()
    onec_d = nc.dram_tensor("onesc", [P, 1], F32R, kind="ExternalInput").ap()
    out_d = nc.dram_tensor("out", [S, NN], F32, kind="ExternalOutput").ap()
    dbg = {}
    if debug_taps:
        for l in range(CONV_DEPTH):
            dbg[f"xsh{l}"] = nc.dram_tensor(
                f"dbg_xsh{l}", [4, P, S], F32, kind="ExternalOutput"
            ).ap()
            dbg[f"ab{l}"] = nc.dram_tensor(
                f"dbg_ab{l}", [2, P, NN], F32, kind="ExternalOutput"
            ).ap()
            dbg[f"ash{l}"] = nc.dram_tensor(
                f"dbg_ash{l}", [P, S], F32, kind="ExternalOutput"
            ).ap()
        dbg["mr"] = nc.dram_tensor("dbg_mr", [2, S, NN], F32, kind="ExternalOutput").ap()
        dbg["ut"] = nc.dram_tensor("dbg_ut", [S, P], F32, kind="ExternalOutput").ap()

    agin = [
        nc.dram_tensor(f"agin{l}", [4, P, S], F32, kind="Internal").ap()
        for l in range(CONV_DEPTH)
    ]
    agout = [
        nc.dram_tensor(
            f"agout{l}", [NCORES, 4, P, S], F32, kind="Internal", addr_space="Shared"
        ).ap()
        for l in range(CONV_DEPTH)
    ]
    # link-phase row staging (DRAM round-trip to re-layout rows to partition 0)
    utd = nc.dram_tensor("utd", [S, P], F32R, kind="Internal").ap()
    md_d = nc.dram_tensor("md", [S, NN], F32R, kind="Internal").ap()
    rd_d = nc.dram_tensor("rd", [S, NN], F32R, kind="Internal").ap()

    with tile.TileContext(nc) as tc:
        with (
            tc.tile_pool(name="wp", bufs=1) as wp,
            tc.tile_pool(name="npl", bufs=5) as npl,
            tc.tile_pool(name="xp", bufs=5) as xp,
            tc.tile_pool(name="wk", bufs=3) as wk,
            tc.tile_pool(name="eb", bufs=2) as ebp,
            tc.tile_pool(name="ps", bufs=4, space="PSUM") as ps,
        ):
            wmat = wp.tile([P, NM * P], F32R)
            biast = wp.tile([P, NB], F32)
            scal = wp.tile([1, 2], F32)
            onesr = wp.tile([1, NN], F32R)
            onesc = wp.tile([P, 1], F32R)
            wtn = wp.tile([1, P], F32R, name="wtn")
            nc.sync.dma_start(wmat[:], wmat_d)
            nc.sync.dma_start(
                wtn[:],
                wvec_d[:, WVEC_SLOTS["wtilneg"] * P : (WVEC_SLOTS["wtilneg"] + 1) * P],
            )
            nc.sync.dma_start(biast[:], bias_d)
            nc.sync.dma_start(scal[:], scal_d)
            nc.sync.dma_start(onesr[:], ones_d)
            nc.sync.dma_start(onesc[:], onec_d)

            def wm(name):  # [128,128] float32r lhsT slot
                return wmat[:, WMAT_SLOTS[name] * P : (WMAT_SLOTS[name] + 1) * P]

            def bi(name):  # [128,1] float32
                return biast[:, BIAS_SLOTS[name] : BIAS_SLOTS[name] + 1]

            ones_f = onesr.bitcast(F32)
            onesc_f = onesc.bitcast(F32)

            def edge_layer(l, nb_for, x_shs):
                """All 4 convs of layer l, software-pipelined at depth 2 over
                (conv, ib) work items so every engine's in-order stream
                interleaves stages of neighbouring items."""
                wvts = {}
                for br, cv in CONVS:
                    tag = f"{l}{br}{cv}"
                    wvt = wk.tile([1, P], F32R, name=f"wvt_{tag}", tag="wvt4")
                    s0 = WVEC_SLOTS[f"we_{tag}"]
                    nc.sync.dma_start(wvt[:], wvec_d[:, s0 * P : (s0 + 1) * P])
                    wvts[(br, cv)] = wvt
                items = [(br, cv, ib) for br, cv in CONVS for ib in range(NBLK)]
                live = {}

                def s0_stage(it):
                    br, cv, ib = it
                    tag = f"{l}{br}{cv}"
                    eidx = EDGE_IDX[(br, cv)]
                    eblk = ebp.tile([1, IB, NN], F32R, name=f"e{tag}_{ib}", tag="eblk")
                    nc.sync.dma_start(
                        eblk[:].rearrange("a b c -> a (b c)"),
                        edg[eidx, ib * IB : (ib + 1) * IB, :]
                        .rearrange("a b -> (a b)")[None, :],
                    )
                    pz0 = ps.tile([P, IB, NN], F32, name=f"z0{tag}{ib}", tag="eps")
                    nb_tile = nb_for(br, cv)
                    for k in range(IB):
                        if nb_tile is not None:
                            nc.tensor.matmul(
                                pz0[:, k, :], wm(f"nb_{tag}"), nb_tile[:],
                                start=True, stop=False,
                            )
                            nc.tensor.matmul(
                                pz0[:, k, :], wvts[(br, cv)][:], eblk[0:1, k, :],
                                start=False, stop=True,
                            )
                        else:
                            nc.tensor.matmul(
                                pz0[:, k, :], wvts[(br, cv)][:], eblk[0:1, k, :],
                                start=True, stop=True,
                            )
                    live[("z0", it)] = pz0

                def s1_stage(it):
                    br, cv, ib = it
                    tag = f"{l}{br}{cv}"
                    pz0 = live.pop(("z0", it))
                    h0 = wk.tile([P, IB, NN], F32R, name=f"h0{tag}{ib}", tag="ha")
                    nc.scalar.activation(
                        h0[:, :, :CA], pz0[:, :, :CA], LRELU,
                        bias=bi(f"b1_{tag}"), scale=1.0, alpha=SLOPE,
                    )
                    nc.vector._custom_dve(
                        _LEAKY_OP, out=h0[:, :, CA:], in0=pz0[:, :, CA:],
                        s0=bi(f"b1_{tag}"), s1=SLOPE,
                    )
                    pz1 = ps.tile([P, IB, NN], F32, name=f"z1{tag}{ib}", tag="eps")
                    for k in range(IB):
                        nc.tensor.matmul(
                            pz1[:, k, :], wm(f"w2_{tag}"), h0[:, k, :],
                            start=True, stop=True,
                        )
                    live[("z1", it)] = pz1

                def s2_stage(it):
                    br, cv, ib = it
                    tag = f"{l}{br}{cv}"
                    pz1 = live.pop(("z1", it))
                    h1 = wk.tile([P, IB, NN], F32R, name=f"h1{tag}{ib}", tag="hb")
                    nc.vector._custom_dve(
                        _LEAKY_OP, out=h1[:], in0=pz1[:], s0=bi(f"b2_{tag}"),
                        s1=SLOPE,
                    )
                    pz2 = ps.tile([P, IB, NN], F32, name=f"z2{tag}{ib}", tag="eps")
                    for k in range(IB):
                        nc.tensor.matmul(
                            pz2[:, k, :], wm(f"w3_{tag}"), h1[:, k, :],
                            start=True, stop=True,
                        )
                    live[("z2", it)] = pz2

                def s3_stage(it):
                    br, cv, ib = it
                    tag = f"{l}{br}{cv}"
                    pz2 = live.pop(("z2", it))
                    junk = wk.tile([P, IB, NN], F32, name=f"j{tag}{ib}", tag="u01")
                    x_sh = x_shs[(br, cv)]
                    for k in range(IB):
                        i = ib * IB + k
                        nc.scalar.activation(
                            junk[:, k, :], pz2[:, k, :], LRELU,
                            bias=bi(f"b3_{tag}"), scale=1.0, alpha=SLOPE,
                            accum_out=x_sh[:, i : i + 1],
                        )

                stages = [s0_stage, s1_stage, s2_stage, s3_stage]
                nst = len(stages)
                for t in range(len(items) + nst - 1):
                    for si in range(nst - 1, -1, -1):
                        j = t - si
                        if 0 <= j < len(items):
                            stages[si](items[j])

            def global_ln_apply(x_full, xhat, tagn):
                """xhat = (x - mean)*rsqrt(var+eps) over the whole [128,512]
                tensor; returns the [128,2] (m,r) broadcast tile for reuse."""
                s12 = wk.tile([P, 2], F32, name=f"s12_{tagn}", tag="s12")
                nc.vector.reduce_sum(s12[:, 0:1], x_full[:], axis=AXX)
                xsq = wk.tile([P, NN], F32, name=f"xsq_{tagn}", tag="u01")
                nc.scalar.activation(
                    xsq[:], x_full[:], SQUARE, bias=0.0, scale=1.0,
                    accum_out=s12[:, 1:2],
                )
                pst = ps.tile([1, 2], F32, name=f"pst_{tagn}", tag="eps")
                nc.tensor.matmul(pst[:], onesc_f, s12[:], start=True, stop=True)
                sc = wk.tile([1, 4], F32, name=f"sc_{tagn}", tag="scs")
                # sc[0]=m, sc[1]=E[x^2]
                nc.vector.tensor_scalar(
                    sc[0:1, 0:2], pst[:], 1.0 / (NN * W), None, MULT
                )
                # sc[2] = var = E[x^2] - m^2 ; sc[3] = sd = sqrt(var + eps)
                nc.scalar.activation(sc[0:1, 2:3], sc[0:1, 0:1], SQUARE)
                nc.vector.tensor_tensor(
                    sc[0:1, 2:3], sc[0:1, 1:2], sc[0:1, 2:3], SUB
                )
                nc.scalar.activation(
                    sc[0:1, 3:4], sc[0:1, 2:3], SQRT, bias=bi("epsv")[0:1, :]
                )
                rin = wk.tile([1, 2], F32, name=f"rin_{tagn}", tag="scs2")
                nc.vector.tensor_copy(rin[0:1, 0:1], sc[0:1, 0:1])  # m
                nc.vector.reciprocal(rin[0:1, 1:2], sc[0:1, 3:4])   # r
                pbc = ps.tile([P, 2], F32, name=f"pbc_{tagn}", tag="eps")
                nc.tensor.matmul(
                    pbc[:], ones_f[0:1, 0:P], rin[:], start=True, stop=True
                )
                mr = npl.tile([P, 2], F32, name=f"mr_{tagn}", tag="mrbc")
                nc.scalar.copy(mr[:], pbc[:])
                nc.vector.tensor_scalar(
                    xhat[:], x_full[:], mr[:, 0:1], mr[:, 1:2], SUB, MULT
                )
                return mr

            def tr_layer(l, br, cv, xhat, out_t):
                tag = f"{l}{br}{cv}"
                pt = ps.tile(
                    [P, xhat.shape[-1]], F32, name=f"ptr_{out_t.name}", tag="eps"
                )
                nc.tensor.matmul(
                    pt[:], wm(f"wtr_{tag}"), xhat[:], start=True, stop=True
                )
                nc.scalar.activation(
                    out_t[:], pt[:], LRELU, bias=bi(f"btr_{tag}"), scale=1.0,
                    alpha=SLOPE,
                )

            def comb_mlp(l, br, c_t, width, sfx):
                h = c_t
                for k in range(3):
                    pc = ps.tile(
                        [P, width], F32, name=f"pcb{k}_{l}{br}{sfx}", tag="eps"
                    )
                    nc.tensor.matmul(
                        pc[:], wm(f"cb{k}_{l}{br}"), h[:], start=True, stop=True
                    )
                    if k == 2:
                        hn = npl.tile(
                            [P, width], F32R,
                            name=f"{br}{l + 1}{sfx}", tag=f"nodes{sfx}",
                        )
                    else:
                        hn = wk.tile(
                            [P, width], F32R,
                            name=f"ch{k}_{l}{br}{sfx}", tag=f"combh{sfx}",
                        )
                    nc.scalar.activation(
                        hn[:], pc[:], LRELU, bias=bi(f"cbb{k}_{l}{br}"),
                        scale=1.0, alpha=SLOPE,
                    )
                    h = hn
                return h

            # ---------------- conv layers ----------------
            a_cur = b_cur = None      # [128, 512] float32r full node features
            a_sh = None               # [128, S] float32r shard chain (a only)
            for l in range(CONV_DEPTH):
                x_shs = {}
                for br, cv in CONVS:
                    x_shs[(br, cv)] = xp.tile(
                        [P, S], F32, name=f"xsh_{l}{br}{cv}", tag="xsh"
                    )

                def nb_for(br, cv, _l=l, _a=a_cur, _b=b_cur):
                    if _l == 0:
                        return None
                    return {"q": {"a": _a, "b": _b},
                            "l": {"a": _b, "b": _a}}[cv][br]

                edge_layer(l, nb_for, x_shs)
                for ci, (br, cv) in enumerate(CONVS):
                    nc.sync.dma_start(agin[l][ci], x_shs[(br, cv)][:])
                if with_collectives:
                    nc.gpsimd.collective_compute(
                        "AllGather",
                        mybir.AluOpType.bypass,
                        replica_groups=[list(range(NCORES))],
                        ins=[agin[l].opt()],
                        outs=[agout[l].opt()],
                    )
                else:
                    nc.sync.dma_start(agout[l][0], agin[l])

                # node phase (redundant on all cores)
                t_full = {}
                mrs = {}
                for ci, (br, cv) in enumerate(CONVS):
                    tag = f"{l}{br}{cv}"
                    x_full = xp.tile([P, NN], F32, name=f"xf_{tag}", tag="xfull")
                    nc.sync.dma_start(
                        x_full[:].rearrange("f (c i) -> f c i", c=NCORES),
                        agout[l][:, ci, :, :].rearrange("c f i -> f c i"),
                    )
                    xhat = wk.tile([P, NN], F32R, name=f"xh_{tag}", tag="hb")
                    mrs[(br, cv)] = global_ln_apply(x_full, xhat, tag)
                    t_t = npl.tile([P, NN], F32R, name=f"t_{tag}", tag="tt")
                    tr_layer(l, br, cv, xhat, t_t)
                    t_full[(br, cv)] = t_t
                new_nodes = {}
                for br in "ab":
                    c_t = npl.tile([P, NN], F32R, name=f"c_{l}{br}", tag="nodes")
                    nc.vector.tensor_tensor(
                        c_t[:],
                        t_full[(br, "q")].bitcast(F32)[:],
                        t_full[(br, "l")].bitcast(F32)[:],
                        ADD,
                    )
                    if l > 0:
                        prev = a_cur if br == "a" else b_cur
                        nc.vector.tensor_tensor(
                            c_t[:], c_t.bitcast(F32)[:],
                            prev.bitcast(F32)[:], ADD,
                        )
                    new_nodes[br] = comb_mlp(l, br, c_t, NN, "")
                # shard chain for branch a (link phase needs per-core rows of a)
                xhq = wk.tile([P, S], F32R, name=f"xhq_sh{l}", tag="shs")
                mr_q = mrs[("a", "q")]
                nc.vector.tensor_scalar(
                    xhq[:], x_shs[("a", "q")][:], mr_q[:, 0:1], mr_q[:, 1:2],
                    SUB, MULT,
                )
                tq_sh = wk.tile([P, S], F32R, name=f"tq_sh{l}", tag="sht")
                tr_layer(l, "a", "q", xhq, tq_sh)
                xhl = wk.tile([P, S], F32R, name=f"xhl_sh{l}", tag="shs")
                mr_l = mrs[("a", "l")]
                nc.vector.tensor_scalar(
                    xhl[:], x_shs[("a", "l")][:], mr_l[:, 0:1], mr_l[:, 1:2],
                    SUB, MULT,
                )
                tl_sh = wk.tile([P, S], F32R, name=f"tl_sh{l}", tag="sht2")
                tr_layer(l, "a", "l", xhl, tl_sh)
                c_sh = npl.tile([P, S], F32R, name=f"csh_{l}", tag="nodesh")
                nc.vector.tensor_tensor(
                    c_sh[:], tq_sh.bitcast(F32)[:],
                    tl_sh.bitcast(F32)[:], ADD,
                )
                if l > 0:
                    nc.vector.tensor_tensor(
                        c_sh[:], c_sh.bitcast(F32)[:],
                        a_sh.bitcast(F32)[:], ADD,
                    )
                a_sh = comb_mlp(l, "a", c_sh, S, "sh")
                a_cur, b_cur = new_nodes["a"], new_nodes["b"]
                if debug_taps:
                    for ci, (br, cv) in enumerate(CONVS):
                        nc.sync.dma_start(dbg[f"xsh{l}"][ci], x_shs[(br, cv)][:])
                    nc.sync.dma_start(dbg[f"ab{l}"][0], a_cur.bitcast(F32)[:])
                    nc.sync.dma_start(dbg[f"ab{l}"][1], b_cur.bitcast(F32)[:])
                    nc.sync.dma_start(dbg[f"ash{l}"], a_sh.bitcast(F32)[:])

            # ---------------- link (pairwise) phase ----------------
            # UT shard [S, 128] = a_sh.T @ W1a' (gamma folded on host)
            put = ps.tile([S, P], F32, name="put", tag="eps")
            nc.tensor.matmul(put[:], a_sh[:, :], wm("w1a"), start=True, stop=True)
            ut_sb = wp.tile([S, P], F32R, name="ut_sb", tag="ut")
            nc.scalar.copy(ut_sb[:], put[:])
            nc.sync.dma_start(utd, ut_sb[:])

            # row sums: sa/qa2 over shard a, sb/qb2 over full b (fp32 matmuls)
            prow_a = ps.tile([1, 2, S], F32, name="prow_a", tag="eps")
            nc.tensor.matmul(
                prow_a[:, 0, :], onesc_f, a_sh.bitcast(F32)[:],
                start=True, stop=True,
            )
            asq = wk.tile([P, S], F32, name="asq", tag="shs")
            nc.scalar.activation(asq[:], a_sh.bitcast(F32)[:], SQUARE)
            nc.tensor.matmul(
                prow_a[:, 1, :], onesc_f, asq[:], start=True, stop=True
            )
            sa_sb = wp.tile([1, 2, S], F32, name="sa_sb", tag="sarow")
            nc.scalar.copy(sa_sb[:], prow_a[:])

            prow_b0 = ps.tile([1, NN], F32, name="prow_b0", tag="eps")
            nc.tensor.matmul(
                prow_b0[:], onesc_f, b_cur.bitcast(F32)[:], start=True, stop=True
            )
            bsq = wk.tile([P, NN], F32, name="bsq", tag="u01")
            nc.scalar.activation(bsq[:], b_cur.bitcast(F32)[:], SQUARE)
            prow_b1 = ps.tile([1, NN], F32, name="prow_b1", tag="eps")
            nc.tensor.matmul(prow_b1[:], onesc_f, bsq[:], start=True, stop=True)
            sb_sb = wp.tile([1, 2, NN], F32, name="sb_sb", tag="sbrow")
            nc.scalar.copy(sb_sb[:, 0, :], prow_b0[:])
            nc.scalar.copy(sb_sb[:, 1, :], prow_b1[:])

            # M, Q [S, 512] ; then R = 1/sqrt(Q/256 - (M/256)^2 + eps)
            pmqs = []
            for t in range(2):
                pmq_t = ps.tile([S, NN], F32, name=f"pmq{t}", tag="eps")
                nc.tensor.matmul(
                    pmq_t[:], sa_sb[0:1, t, :], ones_f[:], start=True, stop=False
                )
                nc.tensor.matmul(
                    pmq_t[:], ones_f[0:1, 0:S], sb_sb[0:1, t, :],
                    start=False, stop=True,
                )
                pmqs.append(pmq_t)
            m_sb = wk.tile([S, NN], F32R, name="m_sb", tag="mlink")
            nc.vector.tensor_scalar(
                m_sb[:], pmqs[0][:], 1.0 / (2 * W), None, MULT
            )
            qn = wk.tile([S, NN], F32, name="qn", tag="ha")
            nc.vector.tensor_scalar(
                qn[:], pmqs[1][:], 1.0 / (2 * W), None, MULT
            )
            msq = wk.tile([S, NN], F32, name="msq", tag="hb")
            nc.scalar.activation(msq[:], m_sb.bitcast(F32)[:], SQUARE)
            nc.vector.tensor_tensor(qn[:], qn[:], msq[:], SUB)
            sd = wk.tile([S, NN], F32, name="sd", tag="hb")
            nc.scalar.activation(sd[:], qn[:], SQRT, bias=bi("epsv")[0:S, :])
            r_sb = wk.tile([S, NN], F32, name="r_sb", tag="rlink")
            nc.vector.reciprocal(r_sb[:], sd[:])
            nc.sync.dma_start(md_d, m_sb[:])
            nc.gpsimd.dma_start(rd_d, r_sb[:])  # fp32 -> f32r cast to DRAM
            if debug_taps:
                nc.sync.dma_start(dbg["mr"][0], m_sb.bitcast(F32)[:])
                nc.sync.dma_start(dbg["mr"][1], r_sb[:])
                nc.sync.dma_start(dbg["ut"], ut_sb.bitcast(F32)[:])

            lv = {}

            def l_s0(ib):
                ublk = ebp.tile([1, IB, P], F32R, name=f"ub{ib}", tag="ublk")
                nc.sync.dma_start(
                    ublk[:].rearrange("a b c -> a (b c)"),
                    utd[ib * IB : (ib + 1) * IB, :].rearrange("a b -> (a b)")[None, :],
                )
                mblk = ebp.tile([1, IB, NN], F32R, name=f"mb{ib}", tag="mblk")
                nc.sync.dma_start(
                    mblk[:].rearrange("a b c -> a (b c)"),
                    md_d[ib * IB : (ib + 1) * IB, :].rearrange("a b -> (a b)")[None, :],
                )
                rblk = ebp.tile([1, IB, NN], F32R, name=f"rb{ib}", tag="rblk")
                nc.sync.dma_start(
                    rblk[:].rearrange("a b c -> a (b c)"),
                    rd_d[ib * IB : (ib + 1) * IB, :].rearrange("a b -> (a b)")[None, :],
                )
                pr = ps.tile([P, IB, NN], F32, name=f"pr{ib}", tag="eps")
                for k in range(IB):
                    nc.tensor.matmul(
                        pr[:, k, :], onesr[0:1, 0:P], rblk[0:1, k, :],
                        start=True, stop=True,
                    )
                rrep = wk.tile([P, IB, NN], F32, name=f"rrep{ib}", tag="u01")
                nc.scalar.copy(rrep[:], pr[:])
                lv[("blk", ib)] = (ublk, mblk, rblk, rrep)

            def l_s1(ib):
                ublk, mblk, rblk, rrep = lv[("blk", ib)]
                pS = ps.tile([P, IB, NN], F32, name=f"pS{ib}", tag="eps")
                for k in range(IB):
                    nc.tensor.matmul(
                        pS[:, k, :], wm("w1b"), b_cur[:], start=True, stop=False
                    )
                    nc.tensor.matmul(
                        pS[:, k, :], ublk[0:1, k, :], onesr[:],
                        start=False, stop=False,
                    )
                    nc.tensor.matmul(
                        pS[:, k, :], wtn[:], mblk[0:1, k, :],
                        start=False, stop=True,
                    )
                z1s = wk.tile([P, IB, NN], F32, name=f"z1s{ib}", tag="z1s")
                nc.vector.tensor_tensor(z1s[:], pS[:], rrep[:], MULT)
                h1L = wk.tile([P, IB, NN], F32R, name=f"h1L{ib}", tag="ha")
                nc.scalar.activation(
                    h1L[:], z1s[:], LRELU, bias=bi("ctil"), scale=1.0, alpha=SLOPE
                )
                lv[("h1", ib)] = h1L

            def l_s2(ib):
                h1L = lv.pop(("h1", ib))
                pz2 = ps.tile([P, IB, NN], F32, name=f"pz2L{ib}", tag="eps")
                for k in range(IB):
                    nc.tensor.matmul(
                        pz2[:, k, :], wm("w2L"), h1L[:, k, :], start=True, stop=True
                    )
                h2L = wk.tile([P, IB, NN], F32R, name=f"h2L{ib}", tag="hb")
                nc.scalar.activation(
                    h2L[:], pz2[:], LRELU, bias=bi("b2L"), scale=1.0, alpha=SLOPE
                )
                lv[("h2", ib)] = h2L

            def l_s3(ib):
                h2L = lv.pop(("h2", ib))
                pz3 = ps.tile([P, IB, NN], F32, name=f"pz3L{ib}", tag="eps")
                for k in range(IB):
                    nc.tensor.matmul(
                        pz3[:, k, :], wm("w3L"), h2L[:, k, :], start=True, stop=True
                    )
                h3L = wk.tile([P, IB, NN], F32R, name=f"h3L{ib}", tag="u01")
                nc.vector._custom_dve(
                    _LEAKY_OP, out=h3L[:], in0=pz3[:], s0=bi("b3L"), s1=SLOPE
                )
                lv[("h3", ib)] = h3L

            def l_s4(ib):
                h3L = lv.pop(("h3", ib))
                po = ps.tile([1, IB, NN], F32, name=f"po{ib}", tag="eps")
                for k in range(IB):
                    nc.tensor.matmul(
                        po[0:1, k, :], wm("w4")[:, 0:1], h3L[:, k, :],
                        start=True, stop=True,
                    )
                oro = wk.tile([1, IB, NN], F32, name=f"oro{ib}", tag="mlink")
                nc.vector.tensor_scalar(oro[:], po[:], scal[0:1, 0:1], None, ADD)
                nc.sync.dma_start(
                    out_d[ib * IB : (ib + 1) * IB, :].rearrange("a b -> (a b)")[None, :],
                    oro[:].rearrange("a b c -> a (b c)"),
                )
                lv.pop(("blk", ib))

            lstages = [l_s0, l_s1, l_s2, l_s3, l_s4]
            for t in range(NBLK + len(lstages) - 1):
                for si in range(len(lstages) - 1, -1, -1):
                    j = t - si
                    if 0 <= j < NBLK:
                        lstages[si](j)

    nc.compile()
    return nc


_NC_CACHE = {}


def _make_runner(nc):
    """jit-once SPMD runner (mirrors bass2jax.run_bass_via_pjrt, cached)."""
    import jax
    import numpy as _np
    from jax.sharding import Mesh, PartitionSpec
    from jax.experimental.shard_map import shard_map
    from concourse import bass2jax, mybir as _mb

    bass2jax.install_neuronx_cc_hook()
    partition_name = nc.partition_id_tensor.name if nc.partition_id_tensor else None
    in_names, out_names, out_avals, zero_outs = [], [], [], []
    for alloc in nc.m.functions[0].allocations:
        if not isinstance(alloc, _mb.MemoryLocationSet):
            continue
        name = alloc.memorylocations[0].name
        if alloc.kind == "ExternalInput":
            if name != partition_name:
                in_names.append(name)
        elif alloc.kind == "ExternalOutput":
            shape = tuple(alloc.tensor_shape)
            dtype = _mb.dt.np(alloc.dtype)
            out_names.append(name)
            out_avals.append(jax.core.ShapedArray(shape, dtype))
            zero_outs.append(_np.zeros(shape, dtype))
    n_params = len(in_names)
    n_outs = len(out_avals)
    all_in_names = list(in_names) + list(out_names)
    if partition_name is not None:
        all_in_names.append(partition_name)

    def _body(*args):
        operands = list(args)
        if partition_name is not None:
            operands.append(bass2jax.partition_id_tensor())
        outs = bass2jax._bass_exec_p.bind(
            *operands,
            out_avals=tuple(out_avals),
            in_names=tuple(all_in_names),
            out_names=tuple(out_names),
            lowering_input_output_aliases=(),
            sim_require_finite=True,
            sim_require_nnan=True,
            nc=nc,
        )
        return tuple(outs)

    devices = jax.devices()[:NCORES]
    mesh = Mesh(_np.asarray(devices), ("core",))
    donate = tuple(range(n_params, n_params + n_outs))
    sharded = jax.jit(
        shard_map(
            _body, mesh=mesh,
            in_specs=(PartitionSpec("core"),) * (n_params + n_outs),
            out_specs=(PartitionSpec("core"),) * n_outs,
            check_rep=False,
        ),
        donate_argnums=donate,
        keep_unused=True,
    )

    def run(in_maps):
        concat_in = [
            _np.concatenate([_np.asarray(in_maps[c][nm]) for c in range(NCORES)], axis=0)
            for nm in in_names
        ]
        concat_zeros = [
            _np.zeros((NCORES * z.shape[0], *z.shape[1:]), z.dtype) for z in zero_outs
        ]
        out_arrs = sharded(*concat_in, *concat_zeros)
        return {
            nm: _np.asarray(out_arrs[i]) for i, nm in enumerate(out_names)
        }

    return run


def _prep_inputs(A, B, L, params):
    A = np.asarray(A, np.float32)
    B = np.asarray(B, np.float32)
    L = np.asarray(L, np.float32)
    wmat, wvec, bias, scal = pack_weights(params)
    onesr = np.ones((1, NN), np.float32)
    onesc = np.ones((P, 1), np.float32)
    LT = np.ascontiguousarray(L.T)
    in_maps = []
    for c in range(NCORES):
        rows = slice(c * S, (c + 1) * S)
        edg = np.stack([A[rows], L[rows], B[rows], LT[rows]], axis=0)
        in_maps.append(
            {
                "edg": np.ascontiguousarray(edg),
                "wmat": wmat,
                "wvec": wvec,
                "bias": bias,
                "scal": scal,
                "onesr": onesr,
                "onesc": onesc,
            }
        )
    return in_maps


def kernel(A, B, L, params):
    if "run" not in _NC_CACHE:
        nc = build_nc(with_collectives=True)
        _NC_CACHE["run"] = _make_runner(nc)
    in_maps = _prep_inputs(A, B, L, params)
    outs = _NC_CACHE["run"](in_maps)
    # concat over cores: out is [8*S, NN] already (core-major rows)
    return outs["out"].astype(np.float32)
